# revision 11
# baseline (speedup 1.0000x reference)
"""Trainium2 Bass kernel for DecoupledMVRowSelfAttnProcessor (bs=6, seq=1024, C=1280,
20 heads, 6 views, row-wise MV attention). Self-contained: builds + compiles an 8-core
SPMD Tile kernel on first call, then runs it via run_bass_kernel_spmd.

v3 design:
  Weights arrive pre-converted to bf16 (host-side). Only THREE collectives:
    - fused base+ref K^T AllGather  (KbrL [2C,TPC] -> shared KbrG [16C,TPC])
    - fused base+ref V   AllGather  (VbrL [TPC,2*CV] -> shared VbrG [TOK,2*CV])
    - MV out rows        AllGather  (OmL [TPC,C] -> shared OmG [TOK,C])
  all with addr_space="Shared" outputs (written once to shared HBM).
  The MV attention branch needs NO collectives: the host passes each core a
  second input shard (hidm_shard) holding exactly the 768 tokens of the 4 image
  rows that core owns (order rl-major, then view, then column), so Q/K/V_mv are
  projected locally and consumed from local DRAM with static offsets.
  V tensors are staged in a padded head-major [V_h | 1] 65-column layout so the
  softmax denominator rides along row 64 of the A@V matmul.

Phases:
  A (token-sharded): X^T/R^T/Xm^T via PE-transpose, 9 QKV projections in bf16;
    base/ref K^T and V gathered (shared outputs); Q^T stays SBUF-resident; MV
    K^T/Q^T/V written to local DRAM only.
  B1 (query-sharded base/ref): softmax over the kv partition axis; O^T
    accumulates SBUF-resident.
  B2 (image-row-sharded MV attention) + MV out-projection, rows AllGathered.
  C (token-sharded): base/ref out-projections + combined bias + MV rows + residual.
"""
import sys
sys.path.insert(0, '/opt/trn_rl_repo')
import contextlib
import math
import numpy as np

import concourse.bass as bass
import concourse.mybir as mybir
from concourse import bacc
from concourse.tile import TileContext
from concourse.bass_utils import run_bass_kernel_spmd
from concourse.masks import make_identity

f32 = mybir.dt.float32
bf16 = mybir.dt.bfloat16
u32 = mybir.dt.uint32
AF = mybir.ActivationFunctionType
ALU = mybir.AluOpType

NCORES = 8
BS, SEQ, C = 6, 1024, 1280
H, HD, NV = 20, 64, 6
IH = IW = 32
TOK = BS * SEQ            # 6144
TPC = TOK // NCORES       # 768
NCI = C // 128            # 10
QC = 256                  # query chunk (always batch-pure)
NQC = TPC // QC           # 3
KT = SEQ // 128           # 8
LKV = NV * IW             # 192
HDP = HD + 1              # 65: [V_h | 1] per-head stride in padded V layout
CV = H * HDP              # 1300: padded V row width
SCALE = 1.0 / math.sqrt(HD)

WNAMES = ["Wq", "Wk", "Wv", "Wq_mv", "Wk_mv", "Wv_mv", "Wq_ref", "Wk_ref",
          "Wv_ref", "Wout", "Wout_mv", "Wout_ref"]

# plan tensor layout (uint32, per core):
#   [0:12]   krow[qc*4+j]  row base (rank*2C) into KbrG (16C, TPC)
#   [12:24]  kcol[qc*4+j]  col base into KbrG
#   [24:27]  vrow[qc]      row base (b*SEQ) into VbrG (TOK, 2CV)
#   [27:51]  crow[tt*4+j]  row base into OmG (TOK, C)
PLAN_LEN = 64

_CACHE = {}


def _build(phases=("A", "G", "B1", "B2", "MP", "C")):
    nc = bacc.Bacc("TRN2", target_bir_lowering=False, debug=False, num_devices=NCORES)

    hid = nc.declare_dram_parameter("hid_shard", [TPC, C], f32, isOutput=False)
    ref = nc.declare_dram_parameter("ref_shard", [TPC, C], f32, isOutput=False)
    hidm = nc.declare_dram_parameter("hidm_shard", [TPC, C], f32, isOutput=False)
    Wn = {n: nc.declare_dram_parameter(n, [C, C], bf16, isOutput=False) for n in WNAMES}
    bsum = nc.declare_dram_parameter("bsum", [1, C], f32, isOutput=False)
    plan = nc.declare_dram_parameter("plan", [1, PLAN_LEN], u32, isOutput=False)
    out = nc.declare_dram_parameter("out_shard", [TPC, C], f32, isOutput=True)

    with TileContext(nc) as tc, contextlib.ExitStack() as stack:
        const = stack.enter_context(tc.tile_pool(name="const", bufs=1))
        resid = stack.enter_context(tc.tile_pool(name="resident", bufs=1))
        dram = stack.enter_context(tc.tile_pool(name="dram", bufs=1, space="DRAM"))

        ident = const.tile([128, 128], bf16)
        make_identity(nc, ident[:])
        ones_row = const.tile([1, 128], bf16)   # lhsT for bias broadcast
        nc.any.memset(ones_row[:], 1.0)
        bsum_bf = const.tile([1, C], bf16)
        nc.gpsimd.dma_start(bsum_bf[:], bsum[:])
        plan_sb = const.tile([1, PLAN_LEN], u32)
        nc.sync.dma_start(plan_sb[:], plan[:])

        def plan_reg(eng, idx, max_val):
            tmp = eng.alloc_register(f"plan_{idx}_{nc.next_id()}")
            eng.reg_load(tmp, plan_sb[0:1, idx:idx + 1])
            return eng.snap(tmp, donate=True, min_val=0, max_val=max_val)

        # resident bf16 tensors: 10 part-tiles of (128, TPC) each
        QbT = [resid.tile([128, TPC], bf16, name=f"QbT{i}", tag=f"QbT{i}") for i in range(NCI)]
        QrT = [resid.tile([128, TPC], bf16, name=f"QrT{i}", tag=f"QrT{i}") for i in range(NCI)]
        ObT = [resid.tile([128, TPC], bf16, name=f"ObT{i}", tag=f"ObT{i}") for i in range(NCI)]
        OrT = [resid.tile([128, TPC], bf16, name=f"OrT{i}", tag=f"OrT{i}") for i in range(NCI)]
        OmT = [resid.tile([128, TPC], bf16, name=f"OmT{i}", tag=f"OmT{i}") for i in range(NCI)]

        KbrL = dram.tile([2 * C, TPC], bf16, tag="KbrL")
        VbrL = dram.tile([TPC, 2 * CV], bf16, tag="VbrL")
        KmL = dram.tile([C, TPC], bf16, tag="KmL")
        QmL = dram.tile([C, TPC], bf16, tag="QmL")
        VmL = dram.tile([TPC, CV], bf16, tag="VmL")
        OmL = dram.tile([TPC, C], bf16, tag="OmL")
        KbrG = dram.tile([NCORES * 2 * C, TPC], bf16, tag="KbrG", addr_space="Shared")
        VbrG = dram.tile([TOK, 2 * CV], bf16, tag="VbrG", addr_space="Shared")
        OmG = dram.tile([TOK, C], bf16, tag="OmG", addr_space="Shared")

        G8 = [list(range(NCORES))]

        def copyback(dst_ap, src_ap, idx):
            if idx % 2:
                nc.vector.tensor_copy(dst_ap, src_ap)
            else:
                nc.scalar.copy(dst_ap, src_ap)

        # ============================ PHASE A ============================
        with tc.tile_pool(name="pXT", bufs=1) as pXT, \
             tc.tile_pool(name="pA", bufs=2) as pA, \
             tc.tile_pool(name="pAwb", bufs=12) as pAwb, \
             tc.tile_pool(name="pAs", bufs=3) as pAs, \
             tc.tile_pool(name="psA", bufs=6, space="PSUM") as psA:

            XT = [pXT.tile([128, TPC], bf16, name=f"XT{i}", tag=f"XT{i}") for i in range(NCI)]
            RT = [pXT.tile([128, TPC], bf16, name=f"RT{i}", tag=f"RT{i}") for i in range(NCI)]
            MT = [pXT.tile([128, TPC], bf16, name=f"MT{i}", tag=f"MT{i}") for i in range(NCI)]
            with tc.tile_pool(name="psT", bufs=2, space="PSUM") as psT:
                for src, dstT in ((hid, XT), (ref, RT), (hidm, MT)):
                    for t in range(TPC // 128):
                        xn = pA.tile([128, C], bf16, tag="xnat")
                        nc.gpsimd.dma_start(xn[:], src[t * 128:(t + 1) * 128, :])
                        for ci in range(NCI):
                            tp = psT.tile([128, 128], bf16, tag="tp")
                            with nc.allow_low_precision(reason="bf16 transpose"):
                                nc.tensor.transpose(tp[:], xn[:, ci * 128:(ci + 1) * 128], ident[:])
                            copyback(dstT[ci][:, t * 128:(t + 1) * 128], tp[:], ci)

            def load_w_bf(wname, tag):
                tiles = []
                for ci in range(NCI):
                    wb = pAwb.tile([128, C], bf16, tag=f"wb_{tag}")
                    nc.scalar.dma_start(wb[:], Wn[wname][ci * 128:(ci + 1) * 128, :])
                    tiles.append(wb)
                return tiles

            def proj_T(wname, XTsrc, dest_sb=None, dest_dram=None, row_off=0):
                wt = load_w_bf(wname, "T")
                for co in range(NCI):
                    if dest_sb is not None:
                        stg = dest_sb[co]
                    else:
                        stg = pAs.tile([128, TPC], bf16, name="stgT", tag="stgT")
                    pss = [psA.tile([128, 512], f32, name=f"psT{k}", tag="psA")
                           for k in range(2)]
                    for ci in range(NCI):
                        for k in range(2):
                            nc.tensor.matmul(
                                pss[k][:, :384], wt[ci][:, co * 128:(co + 1) * 128],
                                XTsrc[ci][:, k * 384:(k + 1) * 384],
                                start=(ci == 0), stop=(ci == NCI - 1))
                    for k in range(2):
                        copyback(stg[:, k * 384:(k + 1) * 384], pss[k][:, :384], k)
                    if dest_dram is not None:
                        nc.sync.dma_start(
                            dest_dram[row_off + co * 128:row_off + (co + 1) * 128, :],
                            stg[:])

            def proj_V(wname, XTsrc, dest_dram, col_off=0):
                # out rows in padded [V_h | 1] layout: head h at cols h*65..h*65+64
                wt = load_w_bf(wname, "N")
                chunks = ((0, 512, 0, 8), (512, 1024, 8, 16), (1024, 1280, 16, 20))
                for t in range(TPC // 128):
                    stg = pAs.tile([128, CV], bf16, tag="stgV")
                    stg_h = stg[:].rearrange("p (h c) -> p h c", c=HDP)
                    nc.any.memset(stg_h[:, :, HD:HDP], 1.0)
                    pss = [psA.tile([128, 512], f32, name=f"psN{k}", tag="psA")
                           for k in range(3)]
                    for ci in range(NCI):
                        for k, (c0, c1, _, _) in enumerate(chunks):
                            nc.tensor.matmul(
                                pss[k][:, :c1 - c0], XTsrc[ci][:, t * 128:(t + 1) * 128],
                                wt[ci][:, c0:c1],
                                start=(ci == 0), stop=(ci == NCI - 1))
                    for k, (c0, c1, h0, h1) in enumerate(chunks):
                        copyback(
                            stg_h[:, h0:h1, 0:HD],
                            pss[k][:, :c1 - c0].rearrange("p (h c) -> p h c", c=HD), k)
                    nc.sync.dma_start(
                        dest_dram[t * 128:(t + 1) * 128, col_off:col_off + CV], stg[:])

            def gather(t_in, t_out):
                if "G" not in phases and "B1" not in phases:
                    return
                nc.gpsimd.collective_compute(
                    "AllGather", ALU.bypass, replica_groups=G8,
                    ins=[t_in[:].opt()], outs=[t_out[:].opt()])

            proj_T("Wk", XT, dest_dram=KbrL, row_off=0)
            proj_T("Wk_ref", RT, dest_dram=KbrL, row_off=C)
            gather(KbrL, KbrG)
            proj_V("Wv", XT, VbrL, col_off=0)
            proj_V("Wv_ref", RT, VbrL, col_off=CV)
            gather(VbrL, VbrG)
            proj_T("Wq", XT, dest_sb=QbT)
            proj_T("Wq_ref", XT, dest_sb=QrT)
            proj_T("Wk_mv", MT, dest_dram=KmL)
            proj_T("Wq_mv", MT, dest_dram=QmL)
            proj_V("Wv_mv", MT, VmL)

        # ============================ PHASE B1: base + ref ============================
        with tc.tile_pool(name="pB", bufs=2) as pB, \
             tc.tile_pool(name="pBk", bufs=2) as pBk, \
             tc.tile_pool(name="pBv", bufs=2) as pBv, \
             tc.tile_pool(name="psB", bufs=3, space="PSUM") as psB, \
             tc.tile_pool(name="psO", bufs=2, space="PSUM") as psO:

            QT_res = {"b": QbT, "r": QrT}
            OT_res = {"b": ObT, "r": OrT}

            for qc in range(NQC if "B1" in phases else 0):
                krows = [plan_reg(nc.sync, qc * 4 + j, 7 * 2 * C) for j in range(4)]
                kcols = [plan_reg(nc.sync, 12 + qc * 4 + j, TPC - QC) for j in range(4)]
                vrow = plan_reg(nc.sync, 24 + qc, TOK - SEQ)
                for tyi, ty in enumerate(("b", "r")):
                    # K^T for all channels: [128, ci, j, QC]; head h lives at
                    # partitions (h%2)*64.. of slab ci=h//2
                    k_sb = pBk.tile([128, NCI, 4, QC], bf16, tag="k_sb")
                    for j in range(4):
                        nc.sync.dma_start(
                            k_sb[:, :, j, :],
                            KbrG[bass.ds(krows[j] + tyi * C, C),
                                 bass.ds(kcols[j], QC)]
                            .rearrange("(ci p) c -> p ci c", p=128))
                    # V rows for this batch in padded [V_h | 1] layout
                    v_sb = pBv.tile([128, KT, CV], bf16, tag="v_sb")
                    nc.sync.dma_start(
                        v_sb[:],
                        VbrG[bass.ds(vrow, SEQ), tyi * CV:(tyi + 1) * CV]
                        .rearrange("(kt p) c -> p kt c", p=128))
                    for hp in range(H // 2):
                        a_sb = {}
                        for g in range(2):
                            s_ps = {}
                            for e in range(2):
                                s_ps[e] = psB.tile([128, 4, QC], f32,
                                                   name=f"s_ps{e}", tag="s_ps")
                            for kk in range(4):
                                kt = g * 4 + kk
                                for e in range(2):
                                    hb = e * 64
                                    nc.tensor.matmul(
                                        s_ps[e][:, kk, :],
                                        k_sb[hb:hb + 64, hp, kt // 2,
                                             (kt % 2) * 128:(kt % 2) * 128 + 128],
                                        QT_res[ty][hp][hb:hb + 64,
                                                       qc * QC:(qc + 1) * QC],
                                        start=True, stop=True)
                            for e in range(2):
                                ab = pB.tile([128, 4, QC], bf16,
                                             name=f"a_sb{g}{e}", tag=f"a_sb{g}{e}")
                                nc.scalar.activation(
                                    ab[:].rearrange("p a b -> p (a b)"),
                                    s_ps[e][:].rearrange("p a b -> p (a b)"),
                                    AF.Exp, scale=SCALE)
                                a_sb[(g, e)] = ab
                        for e in range(2):
                            h = hp * 2 + e
                            o_ps = psO.tile([HDP, QC], f32, tag="o_ps")
                            for kt in range(KT):
                                nc.tensor.matmul(
                                    o_ps[:], v_sb[:, kt, h * HDP:(h + 1) * HDP],
                                    a_sb[(kt // 4, e)][:, kt % 4, :],
                                    start=(kt == 0), stop=(kt == KT - 1))
                            rec = pB.tile([1, QC], f32, tag="rec")
                            nc.vector.reciprocal(rec[:], o_ps[HD:HDP, :])
                            rep = pB.tile([HD, QC], f32, tag="rep")
                            nc.gpsimd.partition_broadcast(rep[:], rec[:])
                            nc.vector.tensor_tensor(
                                out=OT_res[ty][hp][e * 64:e * 64 + 64,
                                                   qc * QC:(qc + 1) * QC],
                                in0=o_ps[0:HD, :], in1=rep[:], op=ALU.mult)

        # ============================ PHASE B2: MV attention ============================
        with tc.tile_pool(name="pM", bufs=2) as pM, \
             tc.tile_pool(name="psM", bufs=2, space="PSUM") as psM:
            for rl in range(4 if "B2" in phases else 0):
                mk = pM.tile([128, NCI, LKV], bf16, tag="mk")
                mq = pM.tile([128, NCI, LKV], bf16, tag="mq")
                for tl, GT in ((mk, KmL), (mq, QmL)):
                    nc.sync.dma_start(
                        tl[:],
                        GT[:, rl * LKV:(rl + 1) * LKV]
                        .rearrange("(ci p) b -> p ci b", p=128))
                mv0 = pM.tile([128, CV], bf16, tag="mv0")
                nc.sync.dma_start(mv0[:], VmL[rl * LKV:rl * LKV + 128, :])
                mv1 = pM.tile([64, CV], bf16, tag="mv1")
                nc.sync.dma_start(mv1[:], VmL[rl * LKV + 128:(rl + 1) * LKV, :])
                for h in range(H):
                    kv = mk[(h % 2) * 64:(h % 2) * 64 + 64, h // 2, :]
                    qv = mq[(h % 2) * 64:(h % 2) * 64 + 64, h // 2, :]
                    s1 = psM.tile([128, LKV], f32, tag="ms1")
                    s2 = psM.tile([64, LKV], f32, tag="ms2")
                    nc.tensor.matmul(s1[:], kv[:, 0:128], qv[:], start=True, stop=True)
                    nc.tensor.matmul(s2[:], kv[:, 128:LKV], qv[:], start=True, stop=True)
                    a1 = pM.tile([128, LKV], bf16, tag="ma1")
                    a2 = pM.tile([64, LKV], bf16, tag="ma2")
                    nc.scalar.activation(a1[:], s1[:], AF.Exp, scale=SCALE)
                    nc.scalar.activation(a2[:], s2[:], AF.Exp, scale=SCALE)
                    o_ps = psM.tile([HDP, LKV], f32, tag="mo")
                    nc.tensor.matmul(o_ps[:], mv0[:, h * HDP:(h + 1) * HDP], a1[:],
                                     start=True, stop=False)
                    nc.tensor.matmul(o_ps[:], mv1[:, h * HDP:(h + 1) * HDP], a2[:],
                                     start=False, stop=True)
                    rec = pM.tile([1, LKV], f32, tag="mrec")
                    nc.vector.reciprocal(rec[:], o_ps[HD:HDP, :])
                    rep = pM.tile([HD, LKV], f32, tag="mrep")
                    nc.gpsimd.partition_broadcast(rep[:], rec[:])
                    nc.vector.tensor_tensor(
                        out=OmT[h // 2][(h % 2) * 64:(h % 2) * 64 + 64,
                                        rl * LKV:(rl + 1) * LKV],
                        in0=o_ps[0:HD, :], in1=rep[:], op=ALU.mult)

        # MV out-projection over local rows, then gather
        with tc.tile_pool(name="pMP", bufs=3) as pMP, \
             tc.tile_pool(name="pMPb", bufs=10) as pMPb, \
             tc.tile_pool(name="psMP", bufs=4, space="PSUM") as psMP:
            wt = []
            for ci in range(NCI if "MP" in phases else 0):
                wb = pMPb.tile([128, C], bf16, tag="mw_b")
                nc.scalar.dma_start(wb[:], Wn["Wout_mv"][ci * 128:(ci + 1) * 128, :])
                wt.append(wb)
            for t in range(TPC // 128 if "MP" in phases else 0):
                stg = pMP.tile([128, C], bf16, tag="m_stg")
                for k, (c0, c1) in enumerate(((0, 512), (512, 1024), (1024, 1280))):
                    ps = psMP.tile([128, 512], f32, tag="psMP")
                    for ci in range(NCI):
                        nc.tensor.matmul(ps[:, :c1 - c0],
                                         OmT[ci][:, t * 128:(t + 1) * 128],
                                         wt[ci][:, c0:c1],
                                         start=(ci == 0), stop=(ci == NCI - 1))
                    copyback(stg[:, c0:c1], ps[:, :c1 - c0], k)
                nc.sync.dma_start(OmL[t * 128:(t + 1) * 128, :], stg[:])
            if "MP" in phases:
                nc.gpsimd.collective_compute(
                    "AllGather", ALU.bypass, replica_groups=G8,
                    ins=[OmL[:].opt()], outs=[OmG[:].opt()])

        # ============================ PHASE C ============================
        with tc.tile_pool(name="pC", bufs=2) as pC, \
             tc.tile_pool(name="pCwb", bufs=10) as pCwb, \
             tc.tile_pool(name="psC", bufs=4, space="PSUM") as psC:
            wts = {}
            for nm in (("Wout", "Wout_ref") if "C" in phases else ()):
                tl = []
                for ci in range(NCI):
                    wb = pCwb.tile([128, C], bf16, tag=f"cw_b_{nm}")
                    nc.scalar.dma_start(wb[:], Wn[nm][ci * 128:(ci + 1) * 128, :])
                    tl.append(wb)
                wts[nm] = tl
            for t in range(TPC // 128 if "C" in phases else 0):
                res_t = pC.tile([128, C], f32, tag="res")
                nc.sync.dma_start(res_t[:], hid[t * 128:(t + 1) * 128, :])
                mv_t = pC.tile([128, C], f32, tag="mvt")
                for j in range(4):
                    mo = plan_reg(nc.gpsimd, 27 + t * 4 + j, TOK - IW)
                    nc.gpsimd.dma_start(mv_t[j * IW:(j + 1) * IW, :],
                                        OmG[bass.ds(mo, IW), :])
                out_t = pC.tile([128, C], f32, tag="outt")
                chunksC = ((0, 512), (512, 1024), (1024, 1280))
                pss = [psC.tile([128, 512], f32, name=f"psC{k}", tag="psC")
                       for k in range(3)]
                first = True
                for srcT, wnm in ((ObT, "Wout"), (OrT, "Wout_ref")):
                    for ci in range(NCI):
                        for k, (c0, c1) in enumerate(chunksC):
                            nc.tensor.matmul(pss[k][:, :c1 - c0],
                                             srcT[ci][:, t * 128:(t + 1) * 128],
                                             wts[wnm][ci][:, c0:c1],
                                             start=first, stop=False)
                        first = False
                for k, (c0, c1) in enumerate(chunksC):
                    nc.tensor.matmul(pss[k][:, :c1 - c0], ones_row[:],
                                     bsum_bf[0:1, c0:c1], start=False, stop=True)
                    t1 = pC.tile([128, 512], f32, tag="t1")
                    nc.vector.tensor_tensor(out=t1[:, :c1 - c0], in0=pss[k][:, :c1 - c0],
                                            in1=res_t[:, c0:c1], op=ALU.add)
                    nc.vector.tensor_tensor(out=out_t[:, c0:c1], in0=t1[:, :c1 - c0],
                                            in1=mv_t[:, c0:c1], op=ALU.add)
                nc.sync.dma_start(out[t * 128:(t + 1) * 128, :], out_t[:])

    nc.compile()
    return nc


def _mv_perm(c):
    idx = []
    for rl in range(4):
        r_gl = 4 * c + rl
        for v in range(NV):
            t0 = v * SEQ + r_gl * IW
            idx.extend(range(t0, t0 + IW))
    return np.asarray(idx)


def _plans():
    plans = []
    for c in range(NCORES):
        p = np.zeros(PLAN_LEN, dtype=np.uint32)
        for qc in range(NQC):
            t0 = c * TPC + qc * QC
            b = t0 // SEQ
            for j in range(4):
                hcol = b * SEQ + QC * j
                rank, col = hcol // TPC, hcol % TPC
                p[qc * 4 + j] = rank * 2 * C
                p[12 + qc * 4 + j] = col
            p[24 + qc] = b * SEQ
        for tt in range(TPC // 128):
            for j in range(4):
                t0 = c * TPC + tt * 128 + j * IW
                v, rem = divmod(t0, SEQ)
                r = rem // IW
                p[27 + tt * 4 + j] = (r // 4) * TPC + (r % 4) * LKV + v * IW
        plans.append(p.reshape(1, PLAN_LEN))
    return plans


def _in_maps(inputs):
    import ml_dtypes
    hid = np.asarray(inputs["hidden_states"], dtype=np.float32).reshape(TOK, C)
    ref = np.asarray(inputs["ref_hidden_states"], dtype=np.float32).reshape(TOK, C)
    bsum = (np.asarray(inputs["bout"]) + np.asarray(inputs["bout_mv"])
            + np.asarray(inputs["bout_ref"])).astype(np.float32).reshape(1, C)
    wbf = {n: np.ascontiguousarray(
        np.asarray(inputs[n], dtype=np.float32).astype(ml_dtypes.bfloat16))
        for n in WNAMES}
    plans = _plans()
    in_maps = []
    for c in range(NCORES):
        m = {
            "hid_shard": np.ascontiguousarray(hid[c * TPC:(c + 1) * TPC]),
            "ref_shard": np.ascontiguousarray(ref[c * TPC:(c + 1) * TPC]),
            "hidm_shard": np.ascontiguousarray(hid[_mv_perm(c)]),
            "bsum": bsum,
            "plan": plans[c],
        }
        m.update(wbf)
        in_maps.append(m)
    return in_maps


def kernel(**inputs):
    if "nc" not in _CACHE:
        _CACHE["nc"] = _build()
    nc = _CACHE["nc"]
    res = run_bass_kernel_spmd(nc, _in_maps(inputs), list(range(NCORES)))
    full = np.concatenate([res.results[c]["out_shard"] for c in range(NCORES)], axis=0)
    return full.reshape(BS, SEQ, C)


if __name__ == "__main__":
    _build()
    print("BUILD OK")
